# revision 10
# baseline (speedup 1.0000x reference)
"""DSVT input layer for Trainium2, 8 NeuronCores.

Outputs (matching reference):
  voxel_feats  : passthrough of the input array.
  inds/mask    : set-partition bookkeeping (argsort/unique/cumsum integer
                 math) — computed on host; ~0.5% of output bytes.
  pos_embeds   : [8, 100000, 192] f32 (614 MB, 99.4% of output bytes) —
                 computed on device. The window position embedding has only
                 12*12 = 144 distinct inputs, so each layer's embedding is a
                 144x192 table lookup. Each core takes 12500 voxels x all 8
                 layers: a one-hot(code) matrix is built on-chip (broadcast
                 matmul + is_equal) and multiplied against the tables
                 (bf16 hi+lo split for f32-exact results), then streamed to
                 DRAM in ~786KB batched DMAs. This keeps the kernel at the
                 HBM write roofline.
"""
import numpy as np
import ml_dtypes
from math import ceil

N_VOXELS = 100000
BATCH = 4
SPARSE_SHAPE = (468, 468, 1)
WIN_SHAPE = (12, 12, 1)
SET_SIZE = 36
N_LAYERS = 8
D_MODEL = 192
SHIFTS = ((0, 0, 0), (6, 6, 0))

N_CORES = 8
PER_CORE = N_VOXELS // N_CORES          # 12500
P = 128
N_TILES = ceil(PER_CORE / P)            # 98
N_PAD = N_TILES * P                     # 12544
G_MAX = 8                               # tiles per staged DMA batch
N_CODES = WIN_SHAPE[0] * WIN_SHAPE[1]   # 144
PAIRS = ((0, 2), (4, 6), (1, 3), (5, 7))  # layer pairs sharing a PSUM bank

BF16 = ml_dtypes.bfloat16


# ---------------------------------------------------------------- host: sets
def _get_window_coors(coors, shift):
    ssx, ssy, ssz = SPARSE_SHAPE
    wx, wy, wz = WIN_SHAPE
    mnx = ceil(ssx / wx) + 1
    mny = ceil(ssy / wy) + 1
    mnz = ceil(ssz / wz) + 1
    max_per_sample = mnx * mny * mnz
    sx, sy, sz = shift
    if ssz == wz:
        sz = 0
    scx = coors[:, 3] + sx
    scy = coors[:, 2] + sy
    scz = coors[:, 1] + sz
    batch_win = coors[:, 0] * max_per_sample + (scx // wx) * mny * mnz + (scy // wy) * mnz + (scz // wz)
    coors_in_win = np.stack([scz % wz, scy % wy, scx % wx], -1)
    return batch_win.astype(np.int64), coors_in_win.astype(np.int32)


def _get_inner_win_inds(win_inds):
    n = win_inds.shape[0]
    order = np.argsort(win_inds, kind="stable")
    s = win_inds[order]
    idx = np.arange(n, dtype=np.int32)
    is_start = np.concatenate([np.ones((1,), bool), s[1:] != s[:-1]])
    run_start = np.maximum.accumulate(np.where(is_start, idx, 0))
    inner = idx - run_start
    out = np.zeros(n, np.int32)
    out[order] = inner
    return out


def _get_set_single_shift(batch_win_inds, coors_in_win):
    wx, wy, wz = WIN_SHAPE
    max_voxel = wx * wy * wz
    n = batch_win_inds.shape[0]
    uniq, contiguous = np.unique(batch_win_inds, return_inverse=True)
    contiguous = contiguous.reshape(-1).astype(np.int32)
    win_num = int(uniq.shape[0])
    count = np.bincount(contiguous, minlength=win_num).astype(np.int32)
    setnum = -(-count // SET_SIZE)
    set_num = int(setnum.sum())
    set_win_inds = np.repeat(np.arange(win_num, dtype=np.int32), setnum)
    offs = np.concatenate([np.zeros(1, np.int64), np.cumsum(setnum)[:-1]]).astype(np.int32)
    set_inds_in_win = np.arange(set_num, dtype=np.int32) - offs[set_win_inds]
    base = set_inds_in_win[:, None] * SET_SIZE + np.arange(SET_SIZE, dtype=np.int32)[None, :]
    sel = (base * count[set_win_inds][:, None]) // (setnum[set_win_inds][:, None] * SET_SIZE)
    sel = sel + set_win_inds[:, None] * max_voxel

    inner = _get_inner_win_inds(contiguous)
    order1 = np.argsort(contiguous.astype(np.int64) * max_voxel + inner, kind="stable")
    c64 = contiguous.astype(np.int64)

    def partition(sort_key):
        order2 = np.argsort(sort_key, kind="stable")
        inner_sorted = np.zeros(n, np.int32)
        inner_sorted[order2] = inner[order1]
        pos_in_batch = inner_sorted.astype(np.int64) + max_voxel * c64
        padding = np.full((win_num * max_voxel,), -1, np.int32)
        padding[pos_in_batch] = np.arange(n, dtype=np.int32)
        return padding[sel]

    key_y = c64 * max_voxel + coors_in_win[:, 1] * wx * wz + coors_in_win[:, 2] * wz + coors_in_win[:, 0]
    key_x = c64 * max_voxel + coors_in_win[:, 2] * wy * wz + coors_in_win[:, 1] * wz + coors_in_win[:, 0]
    inds = np.stack([partition(key_y), partition(key_x)], 0).astype(np.int32)
    prefix = np.roll(inds, 1, axis=-1)
    prefix[:, :, 0] = -1
    mask = inds == prefix
    return inds, mask


# ------------------------------------------------------------- host: tables
def _compute_tables(codes0, codes1, W1, b1, gamma, beta, W2, b2):
    """144-row pos-embed table per layer; BN batch stats via code histograms."""
    wx, wy, _ = WIN_SHAPE
    n = codes0.shape[0]
    k = np.arange(N_CODES)
    loc_tab = np.stack([(k % wx) - wx / 2.0, (k // wx) - wy / 2.0], -1).astype(np.float32)
    counts = [np.bincount(codes0, minlength=N_CODES).astype(np.float64),
              np.bincount(codes1, minlength=N_CODES).astype(np.float64)]
    tables = np.zeros((N_LAYERS, N_CODES, D_MODEL), np.float32)
    for l in range(N_LAYERS):
        cnt = counts[l % 2]
        h = loc_tab @ W1[l] + b1[l]
        h64 = h.astype(np.float64)
        mu = (cnt[:, None] * h64).sum(0) / n
        var = (cnt[:, None] * (h64 - mu) ** 2).sum(0) / n
        mu32 = mu.astype(np.float32)
        var32 = var.astype(np.float32)
        hn = (h - mu32) / np.sqrt(var32 + 1e-5) * gamma[l] + beta[l]
        tables[l] = np.maximum(hn, 0.0) @ W2[l] + b2[l]
    return tables


def _pack_tables(tables):
    """Pack per-layer tables into the SBUF-resident matmul rhs layouts.

    tabs_a: [128, 4 pairs * 2 (hi,lo) * 384] rows k=0..127
    tabs_b: [ 16, 4 pairs * 2 (hi,lo) * 384] rows k=128..143
    where each 384 block is [layer_a 192 | layer_b 192].
    """
    hi = tables.astype(BF16)
    lo = (tables - hi.astype(np.float32)).astype(BF16)
    parts = np.stack([hi, lo], 0)  # [2, 8, 144, 192] bf16
    tabs_a = np.zeros((128, 4, 2, 2, D_MODEL), BF16)
    tabs_b = np.zeros((16, 4, 2, 2, D_MODEL), BF16)
    for p, (la, lb) in enumerate(PAIRS):
        for h in range(2):
            for j, l in enumerate((la, lb)):
                tabs_a[:, p, h, j, :] = parts[h, l, :128, :]
                tabs_b[:, p, h, j, :] = parts[h, l, 128:, :]
    return tabs_a.reshape(128, 4 * 2 * 2 * D_MODEL), tabs_b.reshape(16, 4 * 2 * 2 * D_MODEL)


# ------------------------------------------------------------- device kernel
_NC_CACHE = {}


def _build_nc():
    if "nc" in _NC_CACHE:
        return _NC_CACHE["nc"]
    import concourse.mybir as mybir
    from concourse.bacc import Bacc
    from concourse.tile import TileContext

    dt = mybir.dt
    # Bacc (not plain Bass): its generate_event_semaphores pass splits
    # multi-wait sync_info into the 1-wait-per-instruction form the TRN2
    # ISA requires — walrus rejects plain Bass Tile output for this kernel.
    nc = Bacc()
    # codes row 0: per-tile interleaved (128 shift0 | 128 shift1) voxel codes;
    # row 1: all-ones. bc row 0: ones, row 1: -partition_index. The broadcast
    # matmul bc.T @ codes then yields psum[p, v] = codes[v] - p, so the
    # one-hot compares need only immediate scalars (0 for table rows 0..127,
    # 128 for rows 128..143).
    codes_d = nc.declare_dram_parameter("codes", [2, N_TILES * 256], dt.bfloat16, isOutput=False)
    bc_d = nc.declare_dram_parameter("bc", [2, 128], dt.bfloat16, isOutput=False)
    tabs_a_d = nc.declare_dram_parameter("tabs_a", [128, 3072], dt.bfloat16, isOutput=False)
    tabs_b_d = nc.declare_dram_parameter("tabs_b", [16, 3072], dt.bfloat16, isOutput=False)
    out_d = nc.declare_dram_parameter("out", [N_LAYERS, N_PAD, D_MODEL], dt.float32, isOutput=True)

    with TileContext(nc) as tc:
        with (
            tc.tile_pool(name="const", bufs=1) as const_pool,
            tc.tile_pool(name="stage", bufs=2) as stage_pool,
            tc.tile_pool(name="oh", bufs=4) as oh_pool,
            tc.tile_pool(name="psum_bc", bufs=2, space="PSUM") as psum_bc_pool,
            tc.tile_pool(name="psum_out", bufs=6, space="PSUM") as psum_out_pool,
        ):
            codes_sb = const_pool.tile([2, N_TILES * 256], dt.bfloat16)
            nc.sync.dma_start(codes_sb[:], codes_d[:])
            bc_sb = const_pool.tile([2, 128], dt.bfloat16)
            nc.sync.dma_start(bc_sb[:], bc_d[:])
            tabs_a_sb = const_pool.tile([128, 3072], dt.bfloat16)
            nc.sync.dma_start(tabs_a_sb[:], tabs_a_d[:])
            tabs_b_sb = const_pool.tile([16, 3072], dt.bfloat16)
            nc.sync.dma_start(tabs_b_sb[:], tabs_b_d[:])

            # DRAM out viewed [layer, partition, tile, d] so the SBUF side of
            # the store keeps its partition dim first.
            out_pv = out_d[:].rearrange("l (t p) d -> l p t d", p=P)

            t0 = 0
            while t0 < N_TILES:
                G = min(G_MAX, N_TILES - t0)
                stages = [
                    stage_pool.tile([128, G * 384], dt.float32,
                                    name=f"stage{p}", tag=f"stage{p}")
                    for p in range(4)
                ]
                for g in range(G):
                    t = t0 + g
                    psum_codes = psum_bc_pool.tile([128, 256], dt.float32, tag="bc")
                    nc.tensor.matmul(
                        psum_codes[:], bc_sb[:],
                        codes_sb[:, t * 256:(t + 1) * 256],
                        start=True, stop=True,
                    )
                    ohA = oh_pool.tile([128, 256], dt.bfloat16, tag="ohA")
                    ohB = oh_pool.tile([16, 256], dt.bfloat16, tag="ohB")
                    nc.vector.tensor_scalar(
                        ohA[:], psum_codes[:], 0.0, None,
                        mybir.AluOpType.is_equal,
                    )
                    nc.vector.tensor_scalar(
                        ohB[:], psum_codes[0:16, :], 128.0, None,
                        mybir.AluOpType.is_equal,
                    )
                    for p in range(4):
                        sh = p // 2
                        lhsA = ohA[:, sh * 128:(sh + 1) * 128]
                        lhsB = ohB[:, sh * 128:(sh + 1) * 128]
                        ps = psum_out_pool.tile([128, 384], dt.float32, tag="po")
                        nc.tensor.matmul(ps[:], lhsA, tabs_a_sb[:, (p * 2) * 384:(p * 2 + 1) * 384], start=True, stop=False)
                        nc.tensor.matmul(ps[:], lhsA, tabs_a_sb[:, (p * 2 + 1) * 384:(p * 2 + 2) * 384], start=False, stop=False)
                        nc.tensor.matmul(ps[:], lhsB, tabs_b_sb[:, (p * 2) * 384:(p * 2 + 1) * 384], start=False, stop=False)
                        nc.tensor.matmul(ps[:], lhsB, tabs_b_sb[:, (p * 2 + 1) * 384:(p * 2 + 2) * 384], start=False, stop=True)
                        nc.vector.tensor_copy(stages[p][:, g * 384:(g + 1) * 384], ps[:])
                for p, (la, lb) in enumerate(PAIRS):
                    sv = stages[p][:].rearrange("p (g q d) -> q p g d", q=2, d=D_MODEL)
                    nc.sync.dma_start(out_pv[la, :, t0:t0 + G, :], sv[0])
                    nc.sync.dma_start(out_pv[lb, :, t0:t0 + G, :], sv[1])
                t0 += G

    nc.finalize()
    _NC_CACHE["nc"] = nc
    return nc


def _make_core_inputs(codes0, codes1, tabs_a, tabs_b):
    bc = np.zeros((2, 128), np.float32)
    bc[0, :] = 1.0
    bc[1, :] = -np.arange(128)
    bc = bc.astype(BF16)
    in_maps = []
    for c in range(N_CORES):
        lo_i, hi_i = c * PER_CORE, (c + 1) * PER_CORE
        c0 = np.zeros(N_PAD, np.int32)
        c1 = np.zeros(N_PAD, np.int32)
        c0[:PER_CORE] = codes0[lo_i:hi_i]
        c1[:PER_CORE] = codes1[lo_i:hi_i]
        il = np.stack([c0.reshape(N_TILES, P), c1.reshape(N_TILES, P)], axis=1)
        codes = np.ones((2, N_TILES * 256), np.float32)
        codes[0] = il.reshape(N_TILES * 256)
        in_maps.append({
            "codes": codes.astype(BF16),
            "bc": bc,
            "tabs_a": tabs_a,
            "tabs_b": tabs_b,
        })
    return in_maps


def _run_device(in_maps, trace=False):
    from concourse.bass_utils import run_bass_kernel_spmd
    nc = _build_nc()
    return run_bass_kernel_spmd(nc, in_maps, list(range(N_CORES)), trace=trace)


# ------------------------------------------------------------------- kernel
def kernel(voxel_feats, voxel_coors, W1, b1, gamma, beta, W2, b2, _trace=False, _device_only=False):
    voxel_feats = np.asarray(voxel_feats)
    coors = np.asarray(voxel_coors).astype(np.int64)
    W1, b1 = np.asarray(W1, np.float32), np.asarray(b1, np.float32)
    gamma, beta = np.asarray(gamma, np.float32), np.asarray(beta, np.float32)
    W2, b2 = np.asarray(W2, np.float32), np.asarray(b2, np.float32)

    wx = WIN_SHAPE[0]
    bwi0, ciw0 = _get_window_coors(coors, SHIFTS[0])
    bwi1, ciw1 = _get_window_coors(coors, SHIFTS[1])
    inds0, mask0 = _get_set_single_shift(bwi0, ciw0)
    inds1, mask1 = _get_set_single_shift(bwi1, ciw1)

    codes0 = (ciw0[:, 1] * wx + ciw0[:, 2]).astype(np.int32)
    codes1 = (ciw1[:, 1] * wx + ciw1[:, 2]).astype(np.int32)
    tables = _compute_tables(codes0, codes1, W1, b1, gamma, beta, W2, b2)
    tabs_a, tabs_b = _pack_tables(tables)

    in_maps = _make_core_inputs(codes0, codes1, tabs_a, tabs_b)
    res = _run_device(in_maps, trace=_trace)

    pos_embeds = np.empty((N_LAYERS, N_VOXELS, D_MODEL), np.float32)
    for c in range(N_CORES):
        pos_embeds[:, c * PER_CORE:(c + 1) * PER_CORE, :] = res.results[c]["out"][:, :PER_CORE, :]

    out = (voxel_feats, inds0, mask0, inds1, mask1, pos_embeds)
    if _trace or _device_only:
        return out, res
    return out


# revision 12
# speedup vs baseline: 1.0078x; 1.0078x over previous
"""DSVT input layer for Trainium2, 8 NeuronCores.

Outputs (matching reference):
  voxel_feats  : passthrough of the input array.
  inds/mask    : set-partition bookkeeping (argsort/unique/cumsum integer
                 math) — computed on host; ~0.5% of output bytes.
  pos_embeds   : [8, 100000, 192] f32 (614 MB, 99.4% of output bytes) —
                 computed on device. The window position embedding has only
                 12*12 = 144 distinct inputs, so each layer's embedding is a
                 144x192 table lookup. Each core takes 12500 voxels x all 8
                 layers: a one-hot(code) matrix is built on-chip (broadcast
                 matmul + is_equal) and multiplied against the tables
                 (bf16 hi+lo split for f32-exact results), then streamed to
                 DRAM in ~786KB batched DMAs. This keeps the kernel at the
                 HBM write roofline.
"""
import numpy as np
import ml_dtypes
from math import ceil

N_VOXELS = 100000
BATCH = 4
SPARSE_SHAPE = (468, 468, 1)
WIN_SHAPE = (12, 12, 1)
SET_SIZE = 36
N_LAYERS = 8
D_MODEL = 192
SHIFTS = ((0, 0, 0), (6, 6, 0))

N_CORES = 8
PER_CORE = N_VOXELS // N_CORES          # 12500
P = 128
N_TILES = ceil(PER_CORE / P)            # 98
N_PAD = N_TILES * P                     # 12544
G_MAX = 8                               # tiles per staged DMA batch
N_CODES = WIN_SHAPE[0] * WIN_SHAPE[1]   # 144
PAIRS = ((0, 2), (4, 6), (1, 3), (5, 7))  # layer pairs sharing a PSUM bank

BF16 = ml_dtypes.bfloat16


# ---------------------------------------------------------------- host: sets
def _get_window_coors(coors, shift):
    ssx, ssy, ssz = SPARSE_SHAPE
    wx, wy, wz = WIN_SHAPE
    mnx = ceil(ssx / wx) + 1
    mny = ceil(ssy / wy) + 1
    mnz = ceil(ssz / wz) + 1
    max_per_sample = mnx * mny * mnz
    sx, sy, sz = shift
    if ssz == wz:
        sz = 0
    scx = coors[:, 3] + sx
    scy = coors[:, 2] + sy
    scz = coors[:, 1] + sz
    batch_win = coors[:, 0] * max_per_sample + (scx // wx) * mny * mnz + (scy // wy) * mnz + (scz // wz)
    coors_in_win = np.stack([scz % wz, scy % wy, scx % wx], -1)
    return batch_win.astype(np.int64), coors_in_win.astype(np.int32)


def _get_inner_win_inds(win_inds):
    n = win_inds.shape[0]
    order = np.argsort(win_inds, kind="stable")
    s = win_inds[order]
    idx = np.arange(n, dtype=np.int32)
    is_start = np.concatenate([np.ones((1,), bool), s[1:] != s[:-1]])
    run_start = np.maximum.accumulate(np.where(is_start, idx, 0))
    inner = idx - run_start
    out = np.zeros(n, np.int32)
    out[order] = inner
    return out


def _get_set_single_shift(batch_win_inds, coors_in_win):
    wx, wy, wz = WIN_SHAPE
    max_voxel = wx * wy * wz
    n = batch_win_inds.shape[0]
    uniq, contiguous = np.unique(batch_win_inds, return_inverse=True)
    contiguous = contiguous.reshape(-1).astype(np.int32)
    win_num = int(uniq.shape[0])
    count = np.bincount(contiguous, minlength=win_num).astype(np.int32)
    setnum = -(-count // SET_SIZE)
    set_num = int(setnum.sum())
    set_win_inds = np.repeat(np.arange(win_num, dtype=np.int32), setnum)
    offs = np.concatenate([np.zeros(1, np.int64), np.cumsum(setnum)[:-1]]).astype(np.int32)
    set_inds_in_win = np.arange(set_num, dtype=np.int32) - offs[set_win_inds]
    base = set_inds_in_win[:, None] * SET_SIZE + np.arange(SET_SIZE, dtype=np.int32)[None, :]
    sel = (base * count[set_win_inds][:, None]) // (setnum[set_win_inds][:, None] * SET_SIZE)
    sel = sel + set_win_inds[:, None] * max_voxel

    inner = _get_inner_win_inds(contiguous)
    order1 = np.argsort(contiguous.astype(np.int64) * max_voxel + inner, kind="stable")
    c64 = contiguous.astype(np.int64)

    def partition(sort_key):
        order2 = np.argsort(sort_key, kind="stable")
        inner_sorted = np.zeros(n, np.int32)
        inner_sorted[order2] = inner[order1]
        pos_in_batch = inner_sorted.astype(np.int64) + max_voxel * c64
        padding = np.full((win_num * max_voxel,), -1, np.int32)
        padding[pos_in_batch] = np.arange(n, dtype=np.int32)
        return padding[sel]

    key_y = c64 * max_voxel + coors_in_win[:, 1] * wx * wz + coors_in_win[:, 2] * wz + coors_in_win[:, 0]
    key_x = c64 * max_voxel + coors_in_win[:, 2] * wy * wz + coors_in_win[:, 1] * wz + coors_in_win[:, 0]
    inds = np.stack([partition(key_y), partition(key_x)], 0).astype(np.int32)
    prefix = np.roll(inds, 1, axis=-1)
    prefix[:, :, 0] = -1
    mask = inds == prefix
    return inds, mask


# ------------------------------------------------------------- host: tables
def _compute_tables(codes0, codes1, W1, b1, gamma, beta, W2, b2):
    """144-row pos-embed table per layer; BN batch stats via code histograms."""
    wx, wy, _ = WIN_SHAPE
    n = codes0.shape[0]
    k = np.arange(N_CODES)
    loc_tab = np.stack([(k % wx) - wx / 2.0, (k // wx) - wy / 2.0], -1).astype(np.float32)
    counts = [np.bincount(codes0, minlength=N_CODES).astype(np.float64),
              np.bincount(codes1, minlength=N_CODES).astype(np.float64)]
    tables = np.zeros((N_LAYERS, N_CODES, D_MODEL), np.float32)
    for l in range(N_LAYERS):
        cnt = counts[l % 2]
        h = loc_tab @ W1[l] + b1[l]
        h64 = h.astype(np.float64)
        mu = (cnt[:, None] * h64).sum(0) / n
        var = (cnt[:, None] * (h64 - mu) ** 2).sum(0) / n
        mu32 = mu.astype(np.float32)
        var32 = var.astype(np.float32)
        hn = (h - mu32) / np.sqrt(var32 + 1e-5) * gamma[l] + beta[l]
        tables[l] = np.maximum(hn, 0.0) @ W2[l] + b2[l]
    return tables


def _pack_tables(tables):
    """Pack per-layer tables into the SBUF-resident matmul rhs layouts.

    tabs_a: [128, 4 pairs * 2 (hi,lo) * 384] rows k=0..127
    tabs_b: [ 16, 4 pairs * 2 (hi,lo) * 384] rows k=128..143
    where each 384 block is [layer_a 192 | layer_b 192].
    """
    hi = tables.astype(BF16)
    lo = (tables - hi.astype(np.float32)).astype(BF16)
    parts = np.stack([hi, lo], 0)  # [2, 8, 144, 192] bf16
    tabs_a = np.zeros((128, 4, 2, 2, D_MODEL), BF16)
    tabs_b = np.zeros((16, 4, 2, 2, D_MODEL), BF16)
    for p, (la, lb) in enumerate(PAIRS):
        for h in range(2):
            for j, l in enumerate((la, lb)):
                tabs_a[:, p, h, j, :] = parts[h, l, :128, :]
                tabs_b[:, p, h, j, :] = parts[h, l, 128:, :]
    return tabs_a.reshape(128, 4 * 2 * 2 * D_MODEL), tabs_b.reshape(16, 4 * 2 * 2 * D_MODEL)


# ------------------------------------------------------------- device kernel
_NC_CACHE = {}
import os as _os
_DENSE_TEST = bool(_os.environ.get("KERNEL_DENSE_TEST"))


def _build_nc():
    if "nc" in _NC_CACHE:
        return _NC_CACHE["nc"]
    import concourse.mybir as mybir
    from concourse.bacc import Bacc
    from concourse.tile import TileContext

    dt = mybir.dt
    # Bacc (not plain Bass): its generate_event_semaphores pass splits
    # multi-wait sync_info into the 1-wait-per-instruction form the TRN2
    # ISA requires — walrus rejects plain Bass Tile output for this kernel.
    nc = Bacc()
    # codes row 0: per-tile interleaved (128 shift0 | 128 shift1) voxel codes;
    # row 1: all-ones. bc row 0: ones, row 1: -partition_index. The broadcast
    # matmul bc.T @ codes then yields psum[p, v] = codes[v] - p, so the
    # one-hot compares need only immediate scalars (0 for table rows 0..127,
    # 128 for rows 128..143).
    codes_d = nc.declare_dram_parameter("codes", [2, N_TILES * 256], dt.bfloat16, isOutput=False)
    bc_d = nc.declare_dram_parameter("bc", [2, 128], dt.bfloat16, isOutput=False)
    tabs_a_d = nc.declare_dram_parameter("tabs_a", [128, 3072], dt.bfloat16, isOutput=False)
    tabs_b_d = nc.declare_dram_parameter("tabs_b", [16, 3072], dt.bfloat16, isOutput=False)
    out_d = nc.declare_dram_parameter("out", [N_LAYERS, N_PAD, D_MODEL], dt.float32, isOutput=True)

    with TileContext(nc) as tc:
        with (
            tc.tile_pool(name="const", bufs=1) as const_pool,
            tc.tile_pool(name="stage", bufs=2) as stage_pool,
            tc.tile_pool(name="oh", bufs=4) as oh_pool,
            tc.tile_pool(name="psum_bc", bufs=2, space="PSUM") as psum_bc_pool,
            tc.tile_pool(name="psum_out", bufs=6, space="PSUM") as psum_out_pool,
        ):
            codes_sb = const_pool.tile([2, N_TILES * 256], dt.bfloat16)
            nc.sync.dma_start(codes_sb[:], codes_d[:])
            bc_sb = const_pool.tile([2, 128], dt.bfloat16)
            nc.sync.dma_start(bc_sb[:], bc_d[:])
            tabs_a_sb = const_pool.tile([128, 3072], dt.bfloat16)
            nc.sync.dma_start(tabs_a_sb[:], tabs_a_d[:])
            tabs_b_sb = const_pool.tile([16, 3072], dt.bfloat16)
            nc.sync.dma_start(tabs_b_sb[:], tabs_b_d[:])

            # DRAM out viewed [layer, partition, tile, d] so the SBUF side of
            # the store keeps its partition dim first.
            out_pv = out_d[:].rearrange("l (t p) d -> l p t d", p=P)

            t0 = 0
            while t0 < N_TILES:
                G = min(G_MAX, N_TILES - t0)
                stages = [
                    stage_pool.tile([128, G * 384], dt.float32,
                                    name=f"stage{p}", tag=f"stage{p}")
                    for p in range(4)
                ]
                for g in range(G):
                    t = t0 + g
                    psum_codes = psum_bc_pool.tile([128, 256], dt.float32, tag="bc")
                    nc.tensor.matmul(
                        psum_codes[:], bc_sb[:],
                        codes_sb[:, t * 256:(t + 1) * 256],
                        start=True, stop=True,
                    )
                    ohA = oh_pool.tile([128, 256], dt.bfloat16, tag="ohA")
                    ohB = oh_pool.tile([16, 256], dt.bfloat16, tag="ohB")
                    nc.vector.tensor_scalar(
                        ohA[:], psum_codes[:], 0.0, None,
                        mybir.AluOpType.is_equal,
                    )
                    nc.vector.tensor_scalar(
                        ohB[:], psum_codes[0:16, :], 128.0, None,
                        mybir.AluOpType.is_equal,
                    )
                    for p in range(4):
                        sh = p // 2
                        lhsA = ohA[:, sh * 128:(sh + 1) * 128]
                        lhsB = ohB[:, sh * 128:(sh + 1) * 128]
                        if _DENSE_TEST:  # timing experiment only: wrong results
                            lhsA = tabs_a_sb[:, 0:128]
                            lhsB = tabs_b_sb[:, 0:128]
                        ps = psum_out_pool.tile([128, 384], dt.float32, tag="po")
                        nc.tensor.matmul(ps[:], lhsA, tabs_a_sb[:, (p * 2) * 384:(p * 2 + 1) * 384], start=True, stop=False)
                        nc.tensor.matmul(ps[:], lhsA, tabs_a_sb[:, (p * 2 + 1) * 384:(p * 2 + 2) * 384], start=False, stop=False)
                        nc.tensor.matmul(ps[:], lhsB, tabs_b_sb[:, (p * 2) * 384:(p * 2 + 1) * 384], start=False, stop=False)
                        nc.tensor.matmul(ps[:], lhsB, tabs_b_sb[:, (p * 2 + 1) * 384:(p * 2 + 2) * 384], start=False, stop=True)
                        nc.vector.tensor_copy(stages[p][:, g * 384:(g + 1) * 384], ps[:])
                for p, (la, lb) in enumerate(PAIRS):
                    sv = stages[p][:].rearrange("p (g q d) -> q p g d", q=2, d=D_MODEL)
                    nc.sync.dma_start(out_pv[la, :, t0:t0 + G, :], sv[0])
                    nc.sync.dma_start(out_pv[lb, :, t0:t0 + G, :], sv[1])
                t0 += G

    nc.finalize()
    _NC_CACHE["nc"] = nc
    return nc


def _make_core_inputs(codes0, codes1, tabs_a, tabs_b):
    bc = np.zeros((2, 128), np.float32)
    bc[0, :] = 1.0
    bc[1, :] = -np.arange(128)
    bc = bc.astype(BF16)
    in_maps = []
    for c in range(N_CORES):
        lo_i, hi_i = c * PER_CORE, (c + 1) * PER_CORE
        c0 = np.zeros(N_PAD, np.int32)
        c1 = np.zeros(N_PAD, np.int32)
        c0[:PER_CORE] = codes0[lo_i:hi_i]
        c1[:PER_CORE] = codes1[lo_i:hi_i]
        il = np.stack([c0.reshape(N_TILES, P), c1.reshape(N_TILES, P)], axis=1)
        codes = np.ones((2, N_TILES * 256), np.float32)
        codes[0] = il.reshape(N_TILES * 256)
        in_maps.append({
            "codes": codes.astype(BF16),
            "bc": bc,
            "tabs_a": tabs_a,
            "tabs_b": tabs_b,
        })
    return in_maps


def _run_device(in_maps, trace=False):
    from concourse.bass_utils import run_bass_kernel_spmd
    nc = _build_nc()
    return run_bass_kernel_spmd(nc, in_maps, list(range(N_CORES)), trace=trace)


# ------------------------------------------------------------------- kernel
def kernel(voxel_feats, voxel_coors, W1, b1, gamma, beta, W2, b2, _trace=False, _device_only=False):
    voxel_feats = np.asarray(voxel_feats)
    coors = np.asarray(voxel_coors).astype(np.int64)
    W1, b1 = np.asarray(W1, np.float32), np.asarray(b1, np.float32)
    gamma, beta = np.asarray(gamma, np.float32), np.asarray(beta, np.float32)
    W2, b2 = np.asarray(W2, np.float32), np.asarray(b2, np.float32)

    wx = WIN_SHAPE[0]
    bwi0, ciw0 = _get_window_coors(coors, SHIFTS[0])
    bwi1, ciw1 = _get_window_coors(coors, SHIFTS[1])
    inds0, mask0 = _get_set_single_shift(bwi0, ciw0)
    inds1, mask1 = _get_set_single_shift(bwi1, ciw1)

    codes0 = (ciw0[:, 1] * wx + ciw0[:, 2]).astype(np.int32)
    codes1 = (ciw1[:, 1] * wx + ciw1[:, 2]).astype(np.int32)
    tables = _compute_tables(codes0, codes1, W1, b1, gamma, beta, W2, b2)
    tabs_a, tabs_b = _pack_tables(tables)

    in_maps = _make_core_inputs(codes0, codes1, tabs_a, tabs_b)
    res = _run_device(in_maps, trace=_trace)

    pos_embeds = np.empty((N_LAYERS, N_VOXELS, D_MODEL), np.float32)
    for c in range(N_CORES):
        pos_embeds[:, c * PER_CORE:(c + 1) * PER_CORE, :] = res.results[c]["out"][:, :PER_CORE, :]

    out = (voxel_feats, inds0, mask0, inds1, mask1, pos_embeds)
    if _trace or _device_only:
        return out, res
    return out


# revision 19
# speedup vs baseline: 1.5957x; 1.5834x over previous
"""DSVT input layer for Trainium2, 8 NeuronCores.

Outputs (matching reference):
  voxel_feats  : passthrough of the input array.
  inds/mask    : set-partition bookkeeping (argsort/unique/cumsum integer
                 math) — computed on host; ~0.5% of output bytes.
  pos_embeds   : [8, 100000, 192] f32 (614 MB, 99.4% of output bytes) —
                 computed on device. The window position embedding has only
                 12*12 = 144 distinct inputs, so each layer's embedding is a
                 144x192 table lookup. Each core takes 12500 voxels x all 8
                 layers: a one-hot(code) matrix is built on-chip (broadcast
                 matmul + is_equal) and multiplied against the tables
                 (bf16 hi+lo split for f32-exact results), then streamed to
                 DRAM in ~786KB batched DMAs. This keeps the kernel at the
                 HBM write roofline.
"""
import numpy as np
import ml_dtypes
from math import ceil

N_VOXELS = 100000
BATCH = 4
SPARSE_SHAPE = (468, 468, 1)
WIN_SHAPE = (12, 12, 1)
SET_SIZE = 36
N_LAYERS = 8
D_MODEL = 192
SHIFTS = ((0, 0, 0), (6, 6, 0))

N_CORES = 8
PER_CORE = N_VOXELS // N_CORES          # 12500
P = 128
N_TILES = ceil(PER_CORE / P)            # 98
N_PAD = N_TILES * P                     # 12544
G_MAX = 8                               # tiles per staged DMA batch
N_CODES = WIN_SHAPE[0] * WIN_SHAPE[1]   # 144
PAIRS = ((0, 2), (4, 6), (1, 3), (5, 7))  # layer pairs sharing a PSUM bank

BF16 = ml_dtypes.bfloat16


# ---------------------------------------------------------------- host: sets
def _get_window_coors(coors, shift):
    ssx, ssy, ssz = SPARSE_SHAPE
    wx, wy, wz = WIN_SHAPE
    mnx = ceil(ssx / wx) + 1
    mny = ceil(ssy / wy) + 1
    mnz = ceil(ssz / wz) + 1
    max_per_sample = mnx * mny * mnz
    sx, sy, sz = shift
    if ssz == wz:
        sz = 0
    scx = coors[:, 3] + sx
    scy = coors[:, 2] + sy
    scz = coors[:, 1] + sz
    batch_win = coors[:, 0] * max_per_sample + (scx // wx) * mny * mnz + (scy // wy) * mnz + (scz // wz)
    coors_in_win = np.stack([scz % wz, scy % wy, scx % wx], -1)
    return batch_win.astype(np.int64), coors_in_win.astype(np.int32)


def _get_inner_win_inds(win_inds):
    n = win_inds.shape[0]
    order = np.argsort(win_inds, kind="stable")
    s = win_inds[order]
    idx = np.arange(n, dtype=np.int32)
    is_start = np.concatenate([np.ones((1,), bool), s[1:] != s[:-1]])
    run_start = np.maximum.accumulate(np.where(is_start, idx, 0))
    inner = idx - run_start
    out = np.zeros(n, np.int32)
    out[order] = inner
    return out


def _get_set_single_shift(batch_win_inds, coors_in_win):
    wx, wy, wz = WIN_SHAPE
    max_voxel = wx * wy * wz
    n = batch_win_inds.shape[0]
    uniq, contiguous = np.unique(batch_win_inds, return_inverse=True)
    contiguous = contiguous.reshape(-1).astype(np.int32)
    win_num = int(uniq.shape[0])
    count = np.bincount(contiguous, minlength=win_num).astype(np.int32)
    setnum = -(-count // SET_SIZE)
    set_num = int(setnum.sum())
    set_win_inds = np.repeat(np.arange(win_num, dtype=np.int32), setnum)
    offs = np.concatenate([np.zeros(1, np.int64), np.cumsum(setnum)[:-1]]).astype(np.int32)
    set_inds_in_win = np.arange(set_num, dtype=np.int32) - offs[set_win_inds]
    base = set_inds_in_win[:, None] * SET_SIZE + np.arange(SET_SIZE, dtype=np.int32)[None, :]
    sel = (base * count[set_win_inds][:, None]) // (setnum[set_win_inds][:, None] * SET_SIZE)
    sel = sel + set_win_inds[:, None] * max_voxel

    inner = _get_inner_win_inds(contiguous)
    order1 = np.argsort(contiguous.astype(np.int64) * max_voxel + inner, kind="stable")
    c64 = contiguous.astype(np.int64)

    def partition(sort_key):
        order2 = np.argsort(sort_key, kind="stable")
        inner_sorted = np.zeros(n, np.int32)
        inner_sorted[order2] = inner[order1]
        pos_in_batch = inner_sorted.astype(np.int64) + max_voxel * c64
        padding = np.full((win_num * max_voxel,), -1, np.int32)
        padding[pos_in_batch] = np.arange(n, dtype=np.int32)
        return padding[sel]

    key_y = c64 * max_voxel + coors_in_win[:, 1] * wx * wz + coors_in_win[:, 2] * wz + coors_in_win[:, 0]
    key_x = c64 * max_voxel + coors_in_win[:, 2] * wy * wz + coors_in_win[:, 1] * wz + coors_in_win[:, 0]
    inds = np.stack([partition(key_y), partition(key_x)], 0).astype(np.int32)
    prefix = np.roll(inds, 1, axis=-1)
    prefix[:, :, 0] = -1
    mask = inds == prefix
    return inds, mask


# ------------------------------------------------------------- host: tables
def _compute_tables(codes0, codes1, W1, b1, gamma, beta, W2, b2):
    """144-row pos-embed table per layer; BN batch stats via code histograms."""
    wx, wy, _ = WIN_SHAPE
    n = codes0.shape[0]
    k = np.arange(N_CODES)
    loc_tab = np.stack([(k % wx) - wx / 2.0, (k // wx) - wy / 2.0], -1).astype(np.float32)
    counts = [np.bincount(codes0, minlength=N_CODES).astype(np.float64),
              np.bincount(codes1, minlength=N_CODES).astype(np.float64)]
    tables = np.zeros((N_LAYERS, N_CODES, D_MODEL), np.float32)
    for l in range(N_LAYERS):
        cnt = counts[l % 2]
        h = loc_tab @ W1[l] + b1[l]
        h64 = h.astype(np.float64)
        mu = (cnt[:, None] * h64).sum(0) / n
        var = (cnt[:, None] * (h64 - mu) ** 2).sum(0) / n
        mu32 = mu.astype(np.float32)
        var32 = var.astype(np.float32)
        hn = (h - mu32) / np.sqrt(var32 + 1e-5) * gamma[l] + beta[l]
        tables[l] = np.maximum(hn, 0.0) @ W2[l] + b2[l]
    return tables


def _pack_tables(tables):
    """Pack per-layer f32 tables into the SBUF-resident matmul rhs layouts.

    The gather matmuls run in float32r: exact for one-hot lhsT (the dropped
    lo*lo cross term is zero when one operand is exactly 1.0), and the cost
    model gives 1 cycle/row for moving dim >= 256 — so no bf16 hi/lo split
    is needed.

    tabs_a: [128, 4 pairs * 384] rows k=0..127
    tabs_b: [ 16, 4 pairs * 384] rows k=128..143
    where each 384 block is [layer_a 192 | layer_b 192].
    """
    hi = tables.astype(BF16).astype(np.float32)
    rounded = hi + (tables - hi).astype(BF16).astype(np.float32)  # fp32r grid
    tabs_a = np.zeros((128, 4, 2, D_MODEL), np.float32)
    tabs_b = np.zeros((16, 4, 2, D_MODEL), np.float32)
    for p, (la, lb) in enumerate(PAIRS):
        for j, l in enumerate((la, lb)):
            tabs_a[:, p, j, :] = rounded[l, :128, :]
            tabs_b[:, p, j, :] = rounded[l, 128:, :]
    return tabs_a.reshape(128, 4 * 2 * D_MODEL), tabs_b.reshape(16, 4 * 2 * D_MODEL)


# ------------------------------------------------------------- device kernel
_NC_CACHE = {}
import os as _os
_DENSE_TEST = bool(_os.environ.get("KERNEL_DENSE_TEST"))


def _build_nc():
    if "nc" in _NC_CACHE:
        return _NC_CACHE["nc"]
    import concourse.mybir as mybir
    from concourse.bacc import Bacc
    from concourse.tile import TileContext

    dt = mybir.dt
    # Bacc (not plain Bass): its generate_event_semaphores pass splits
    # multi-wait sync_info into the 1-wait-per-instruction form the TRN2
    # ISA requires — walrus rejects plain Bass Tile output for this kernel.
    nc = Bacc()
    # codes row 0: per-tile interleaved (128 shift0 | 128 shift1) voxel codes;
    # row 1: all-ones. bc row 0: ones, row 1: -partition_index. The broadcast
    # matmul bc.T @ codes then yields psum[p, v] = codes[v] - p, so the
    # one-hot compares need only immediate scalars (0 for table rows 0..127,
    # 128 for rows 128..143).
    codes_d = nc.declare_dram_parameter("codes", [2, N_TILES * 256], dt.bfloat16, isOutput=False)
    bc_d = nc.declare_dram_parameter("bc", [2, 128], dt.bfloat16, isOutput=False)
    tabs_a_d = nc.declare_dram_parameter("tabs_a", [128, 1536], dt.float32r, isOutput=False)
    tabs_b_d = nc.declare_dram_parameter("tabs_b", [16, 1536], dt.float32r, isOutput=False)
    out_d = nc.declare_dram_parameter("out", [N_LAYERS, N_PAD, D_MODEL], dt.float32, isOutput=True)

    with TileContext(nc) as tc:
        with (
            tc.tile_pool(name="const", bufs=1) as const_pool,
            tc.tile_pool(name="stage", bufs=2) as stage_pool,
            tc.tile_pool(name="oh", bufs=4) as oh_pool,
            tc.tile_pool(name="psum_bc", bufs=2, space="PSUM") as psum_bc_pool,
            tc.tile_pool(name="psum_out", bufs=6, space="PSUM") as psum_out_pool,
        ):
            codes_sb = const_pool.tile([2, N_TILES * 256], dt.bfloat16)
            nc.sync.dma_start(codes_sb[:], codes_d[:])
            bc_sb = const_pool.tile([2, 128], dt.bfloat16)
            nc.sync.dma_start(bc_sb[:], bc_d[:])
            tabs_a_sb = const_pool.tile([128, 1536], dt.float32r)
            nc.sync.dma_start(tabs_a_sb[:], tabs_a_d[:])
            tabs_b_sb = const_pool.tile([16, 1536], dt.float32r)
            nc.sync.dma_start(tabs_b_sb[:], tabs_b_d[:])

            # DRAM out viewed [layer, partition, tile, d] so the SBUF side of
            # the store keeps its partition dim first.
            out_pv = out_d[:].rearrange("l (t p) d -> l p t d", p=P)

            t0 = 0
            while t0 < N_TILES:
                G = min(G_MAX, N_TILES - t0)
                stages = [
                    stage_pool.tile([128, G * 384], dt.float32,
                                    name=f"stage{p}", tag=f"stage{p}")
                    for p in range(4)
                ]
                for g in range(G):
                    t = t0 + g
                    psum_codes = psum_bc_pool.tile([128, 256], dt.float32, tag="bc")
                    nc.tensor.matmul(
                        psum_codes[:], bc_sb[:],
                        codes_sb[:, t * 256:(t + 1) * 256],
                        start=True, stop=True,
                    )
                    ohA = oh_pool.tile([128, 256], dt.float32r, tag="ohA")
                    ohB = oh_pool.tile([16, 256], dt.float32r, tag="ohB")
                    nc.vector.tensor_scalar(
                        ohA[:], psum_codes[:], 0.0, None,
                        mybir.AluOpType.is_equal,
                    )
                    nc.vector.tensor_scalar(
                        ohB[:], psum_codes[0:16, :], 128.0, None,
                        mybir.AluOpType.is_equal,
                    )
                    for p in range(4):
                        sh = p // 2
                        lhsA = ohA[:, sh * 128:(sh + 1) * 128]
                        lhsB = ohB[:, sh * 128:(sh + 1) * 128]
                        ps = psum_out_pool.tile([128, 384], dt.float32, tag="po")
                        nc.tensor.matmul(ps[:], lhsA, tabs_a_sb[:, p * 384:(p + 1) * 384], start=True, stop=False)
                        nc.tensor.matmul(ps[:], lhsB, tabs_b_sb[:, p * 384:(p + 1) * 384], start=False, stop=True)
                        nc.vector.tensor_copy(stages[p][:, g * 384:(g + 1) * 384], ps[:])
                for p, (la, lb) in enumerate(PAIRS):
                    sv = stages[p][:].rearrange("p (g q d) -> q p g d", q=2, d=D_MODEL)
                    nc.sync.dma_start(out_pv[la, :, t0:t0 + G, :], sv[0])
                    nc.sync.dma_start(out_pv[lb, :, t0:t0 + G, :], sv[1])
                t0 += G

    nc.finalize()
    _NC_CACHE["nc"] = nc
    return nc


def _make_core_inputs(codes0, codes1, tabs_a, tabs_b):
    bc = np.zeros((2, 128), np.float32)
    bc[0, :] = 1.0
    bc[1, :] = -np.arange(128)
    bc = bc.astype(BF16)
    in_maps = []
    for c in range(N_CORES):
        lo_i, hi_i = c * PER_CORE, (c + 1) * PER_CORE
        c0 = np.zeros(N_PAD, np.int32)
        c1 = np.zeros(N_PAD, np.int32)
        c0[:PER_CORE] = codes0[lo_i:hi_i]
        c1[:PER_CORE] = codes1[lo_i:hi_i]
        il = np.stack([c0.reshape(N_TILES, P), c1.reshape(N_TILES, P)], axis=1)
        codes = np.ones((2, N_TILES * 256), np.float32)
        codes[0] = il.reshape(N_TILES * 256)
        in_maps.append({
            "codes": codes.astype(BF16),
            "bc": bc,
            "tabs_a": tabs_a,
            "tabs_b": tabs_b,
        })
    return in_maps


def _run_device(in_maps, trace=False):
    from concourse.bass_utils import run_bass_kernel_spmd
    nc = _build_nc()
    return run_bass_kernel_spmd(nc, in_maps, list(range(N_CORES)), trace=trace)


# ------------------------------------------------------------------- kernel
def kernel(voxel_feats, voxel_coors, W1, b1, gamma, beta, W2, b2, _trace=False, _device_only=False):
    voxel_feats = np.asarray(voxel_feats)
    coors = np.asarray(voxel_coors).astype(np.int64)
    W1, b1 = np.asarray(W1, np.float32), np.asarray(b1, np.float32)
    gamma, beta = np.asarray(gamma, np.float32), np.asarray(beta, np.float32)
    W2, b2 = np.asarray(W2, np.float32), np.asarray(b2, np.float32)

    wx = WIN_SHAPE[0]
    bwi0, ciw0 = _get_window_coors(coors, SHIFTS[0])
    bwi1, ciw1 = _get_window_coors(coors, SHIFTS[1])
    inds0, mask0 = _get_set_single_shift(bwi0, ciw0)
    inds1, mask1 = _get_set_single_shift(bwi1, ciw1)

    codes0 = (ciw0[:, 1] * wx + ciw0[:, 2]).astype(np.int32)
    codes1 = (ciw1[:, 1] * wx + ciw1[:, 2]).astype(np.int32)
    tables = _compute_tables(codes0, codes1, W1, b1, gamma, beta, W2, b2)
    tabs_a, tabs_b = _pack_tables(tables)

    in_maps = _make_core_inputs(codes0, codes1, tabs_a, tabs_b)
    res = _run_device(in_maps, trace=_trace)

    pos_embeds = np.empty((N_LAYERS, N_VOXELS, D_MODEL), np.float32)
    for c in range(N_CORES):
        pos_embeds[:, c * PER_CORE:(c + 1) * PER_CORE, :] = res.results[c]["out"][:, :PER_CORE, :]

    out = (voxel_feats, inds0, mask0, inds1, mask1, pos_embeds)
    if _trace or _device_only:
        return out, res
    return out


# revision 20
# speedup vs baseline: 1.5971x; 1.0009x over previous
"""DSVT input layer for Trainium2, 8 NeuronCores.

Outputs (matching reference):
  voxel_feats  : passthrough of the input array.
  inds/mask    : set-partition bookkeeping (argsort/unique/cumsum integer
                 math) — computed on host; ~0.5% of output bytes.
  pos_embeds   : [8, 100000, 192] f32 (614 MB, 99.4% of output bytes) —
                 computed on device. The window position embedding has only
                 12*12 = 144 distinct inputs, so each layer's embedding is a
                 144x192 table lookup. Each core takes 12500 voxels x all 8
                 layers: a one-hot(code) matrix is built on-chip (broadcast
                 matmul + is_equal) and multiplied against the tables
                 (bf16 hi+lo split for f32-exact results), then streamed to
                 DRAM in ~786KB batched DMAs. This keeps the kernel at the
                 HBM write roofline.
"""
import numpy as np
import ml_dtypes
from math import ceil

N_VOXELS = 100000
BATCH = 4
SPARSE_SHAPE = (468, 468, 1)
WIN_SHAPE = (12, 12, 1)
SET_SIZE = 36
N_LAYERS = 8
D_MODEL = 192
SHIFTS = ((0, 0, 0), (6, 6, 0))

N_CORES = 8
PER_CORE = N_VOXELS // N_CORES          # 12500
P = 128
N_TILES = ceil(PER_CORE / P)            # 98
N_PAD = N_TILES * P                     # 12544
G_MAX = 8                               # tiles per staged DMA batch
N_CODES = WIN_SHAPE[0] * WIN_SHAPE[1]   # 144
PAIRS = ((0, 2), (4, 6), (1, 3), (5, 7))  # layer pairs sharing a PSUM bank

BF16 = ml_dtypes.bfloat16


# ---------------------------------------------------------------- host: sets
def _get_window_coors(coors, shift):
    ssx, ssy, ssz = SPARSE_SHAPE
    wx, wy, wz = WIN_SHAPE
    mnx = ceil(ssx / wx) + 1
    mny = ceil(ssy / wy) + 1
    mnz = ceil(ssz / wz) + 1
    max_per_sample = mnx * mny * mnz
    sx, sy, sz = shift
    if ssz == wz:
        sz = 0
    scx = coors[:, 3] + sx
    scy = coors[:, 2] + sy
    scz = coors[:, 1] + sz
    batch_win = coors[:, 0] * max_per_sample + (scx // wx) * mny * mnz + (scy // wy) * mnz + (scz // wz)
    coors_in_win = np.stack([scz % wz, scy % wy, scx % wx], -1)
    return batch_win.astype(np.int64), coors_in_win.astype(np.int32)


def _get_inner_win_inds(win_inds):
    n = win_inds.shape[0]
    order = np.argsort(win_inds, kind="stable")
    s = win_inds[order]
    idx = np.arange(n, dtype=np.int32)
    is_start = np.concatenate([np.ones((1,), bool), s[1:] != s[:-1]])
    run_start = np.maximum.accumulate(np.where(is_start, idx, 0))
    inner = idx - run_start
    out = np.zeros(n, np.int32)
    out[order] = inner
    return out


def _get_set_single_shift(batch_win_inds, coors_in_win):
    wx, wy, wz = WIN_SHAPE
    max_voxel = wx * wy * wz
    n = batch_win_inds.shape[0]
    uniq, contiguous = np.unique(batch_win_inds, return_inverse=True)
    contiguous = contiguous.reshape(-1).astype(np.int32)
    win_num = int(uniq.shape[0])
    count = np.bincount(contiguous, minlength=win_num).astype(np.int32)
    setnum = -(-count // SET_SIZE)
    set_num = int(setnum.sum())
    set_win_inds = np.repeat(np.arange(win_num, dtype=np.int32), setnum)
    offs = np.concatenate([np.zeros(1, np.int64), np.cumsum(setnum)[:-1]]).astype(np.int32)
    set_inds_in_win = np.arange(set_num, dtype=np.int32) - offs[set_win_inds]
    base = set_inds_in_win[:, None] * SET_SIZE + np.arange(SET_SIZE, dtype=np.int32)[None, :]
    sel = (base * count[set_win_inds][:, None]) // (setnum[set_win_inds][:, None] * SET_SIZE)
    sel = sel + set_win_inds[:, None] * max_voxel

    inner = _get_inner_win_inds(contiguous)
    order1 = np.argsort(contiguous.astype(np.int64) * max_voxel + inner, kind="stable")
    c64 = contiguous.astype(np.int64)

    def partition(sort_key):
        order2 = np.argsort(sort_key, kind="stable")
        inner_sorted = np.zeros(n, np.int32)
        inner_sorted[order2] = inner[order1]
        pos_in_batch = inner_sorted.astype(np.int64) + max_voxel * c64
        padding = np.full((win_num * max_voxel,), -1, np.int32)
        padding[pos_in_batch] = np.arange(n, dtype=np.int32)
        return padding[sel]

    key_y = c64 * max_voxel + coors_in_win[:, 1] * wx * wz + coors_in_win[:, 2] * wz + coors_in_win[:, 0]
    key_x = c64 * max_voxel + coors_in_win[:, 2] * wy * wz + coors_in_win[:, 1] * wz + coors_in_win[:, 0]
    inds = np.stack([partition(key_y), partition(key_x)], 0).astype(np.int32)
    prefix = np.roll(inds, 1, axis=-1)
    prefix[:, :, 0] = -1
    mask = inds == prefix
    return inds, mask


# ------------------------------------------------------------- host: tables
def _compute_tables(codes0, codes1, W1, b1, gamma, beta, W2, b2):
    """144-row pos-embed table per layer; BN batch stats via code histograms."""
    wx, wy, _ = WIN_SHAPE
    n = codes0.shape[0]
    k = np.arange(N_CODES)
    loc_tab = np.stack([(k % wx) - wx / 2.0, (k // wx) - wy / 2.0], -1).astype(np.float32)
    counts = [np.bincount(codes0, minlength=N_CODES).astype(np.float64),
              np.bincount(codes1, minlength=N_CODES).astype(np.float64)]
    tables = np.zeros((N_LAYERS, N_CODES, D_MODEL), np.float32)
    for l in range(N_LAYERS):
        cnt = counts[l % 2]
        h = loc_tab @ W1[l] + b1[l]
        h64 = h.astype(np.float64)
        mu = (cnt[:, None] * h64).sum(0) / n
        var = (cnt[:, None] * (h64 - mu) ** 2).sum(0) / n
        mu32 = mu.astype(np.float32)
        var32 = var.astype(np.float32)
        hn = (h - mu32) / np.sqrt(var32 + 1e-5) * gamma[l] + beta[l]
        tables[l] = np.maximum(hn, 0.0) @ W2[l] + b2[l]
    return tables


def _pack_tables(tables):
    """Pack per-layer f32 tables into the SBUF-resident matmul rhs layouts.

    The gather matmuls run in float32r: exact for one-hot lhsT (the dropped
    lo*lo cross term is zero when one operand is exactly 1.0), and the cost
    model gives 1 cycle/row for moving dim >= 256 — so no bf16 hi/lo split
    is needed.

    tabs_a: [128, 4 pairs * 384] rows k=0..127
    tabs_b: [ 16, 4 pairs * 384] rows k=128..143
    where each 384 block is [layer_a 192 | layer_b 192].
    """
    hi = tables.astype(BF16).astype(np.float32)
    rounded = hi + (tables - hi).astype(BF16).astype(np.float32)  # fp32r grid
    tabs_a = np.zeros((128, 4, 2, D_MODEL), np.float32)
    tabs_b = np.zeros((16, 4, 2, D_MODEL), np.float32)
    for p, (la, lb) in enumerate(PAIRS):
        for j, l in enumerate((la, lb)):
            tabs_a[:, p, j, :] = rounded[l, :128, :]
            tabs_b[:, p, j, :] = rounded[l, 128:, :]
    return tabs_a.reshape(128, 4 * 2 * D_MODEL), tabs_b.reshape(16, 4 * 2 * D_MODEL)


# ------------------------------------------------------------- device kernel
_NC_CACHE = {}
import os as _os
_DENSE_TEST = bool(_os.environ.get("KERNEL_DENSE_TEST"))


def _build_nc():
    if "nc" in _NC_CACHE:
        return _NC_CACHE["nc"]
    import concourse.mybir as mybir
    from concourse.bacc import Bacc
    from concourse.tile import TileContext

    dt = mybir.dt
    # Bacc (not plain Bass): its generate_event_semaphores pass splits
    # multi-wait sync_info into the 1-wait-per-instruction form the TRN2
    # ISA requires — walrus rejects plain Bass Tile output for this kernel.
    nc = Bacc()
    # codes row 0: per-tile interleaved (128 shift0 | 128 shift1) voxel codes;
    # row 1: all-ones. bc row 0: ones, row 1: -partition_index. The broadcast
    # matmul bc.T @ codes then yields psum[p, v] = codes[v] - p, so the
    # one-hot compares need only immediate scalars (0 for table rows 0..127,
    # 128 for rows 128..143).
    codes_d = nc.declare_dram_parameter("codes", [2, N_TILES * 256], dt.bfloat16, isOutput=False)
    bc_d = nc.declare_dram_parameter("bc", [2, 128], dt.bfloat16, isOutput=False)
    tabs_a_d = nc.declare_dram_parameter("tabs_a", [128, 1536], dt.float32r, isOutput=False)
    tabs_b_d = nc.declare_dram_parameter("tabs_b", [16, 1536], dt.float32r, isOutput=False)
    out_d = nc.declare_dram_parameter("out", [N_LAYERS, N_PAD, D_MODEL], dt.float32, isOutput=True)

    with TileContext(nc) as tc:
        with (
            tc.tile_pool(name="const", bufs=1) as const_pool,
            tc.tile_pool(name="stage", bufs=2) as stage_pool,
            tc.tile_pool(name="oh", bufs=4) as oh_pool,
            tc.tile_pool(name="psum_bc", bufs=2, space="PSUM") as psum_bc_pool,
            tc.tile_pool(name="psum_out", bufs=6, space="PSUM") as psum_out_pool,
        ):
            codes_sb = const_pool.tile([2, N_TILES * 256], dt.bfloat16)
            nc.sync.dma_start(codes_sb[:], codes_d[:])
            bc_sb = const_pool.tile([2, 128], dt.bfloat16)
            nc.sync.dma_start(bc_sb[:], bc_d[:])
            tabs_a_sb = const_pool.tile([128, 1536], dt.float32r)
            nc.sync.dma_start(tabs_a_sb[:], tabs_a_d[:])
            tabs_b_sb = const_pool.tile([16, 1536], dt.float32r)
            nc.sync.dma_start(tabs_b_sb[:], tabs_b_d[:])

            # DRAM out viewed [layer, partition, tile, d] so the SBUF side of
            # the store keeps its partition dim first.
            out_pv = out_d[:].rearrange("l (t p) d -> l p t d", p=P)

            t0 = 0
            while t0 < N_TILES:
                G = min(G_MAX, N_TILES - t0)
                stages = [
                    stage_pool.tile([128, G * 384], dt.float32,
                                    name=f"stage{p}", tag=f"stage{p}")
                    for p in range(4)
                ]
                for g in range(G):
                    t = t0 + g
                    psum_codes = psum_bc_pool.tile([128, 256], dt.float32, tag="bc")
                    nc.tensor.matmul(
                        psum_codes[:], bc_sb[:],
                        codes_sb[:, t * 256:(t + 1) * 256],
                        start=True, stop=True,
                    )
                    ohA = oh_pool.tile([128, 256], dt.float32r, tag="ohA")
                    ohB = oh_pool.tile([16, 256], dt.float32r, tag="ohB")
                    nc.vector.tensor_scalar(
                        ohA[:], psum_codes[:], 0.0, None,
                        mybir.AluOpType.is_equal,
                    )
                    nc.vector.tensor_scalar(
                        ohB[:], psum_codes[0:16, :], 128.0, None,
                        mybir.AluOpType.is_equal,
                    )
                    # Order matmuls into same-lhsT runs (pairs 2p, 2p+1 share
                    # the shift-s one-hot): avoids a weight switch per matmul.
                    pss = [psum_out_pool.tile([128, 384], dt.float32, tag="po",
                                              name=f"po{p}") for p in range(4)]
                    for sh in range(2):
                        lhsA = ohA[:, sh * 128:(sh + 1) * 128]
                        lhsB = ohB[:, sh * 128:(sh + 1) * 128]
                        p0, p1 = 2 * sh, 2 * sh + 1
                        nc.tensor.matmul(pss[p0][:], lhsA, tabs_a_sb[:, p0 * 384:(p0 + 1) * 384], start=True, stop=False)
                        nc.tensor.matmul(pss[p1][:], lhsA, tabs_a_sb[:, p1 * 384:(p1 + 1) * 384], start=True, stop=False)
                        nc.tensor.matmul(pss[p0][:], lhsB, tabs_b_sb[:, p0 * 384:(p0 + 1) * 384], start=False, stop=True)
                        nc.tensor.matmul(pss[p1][:], lhsB, tabs_b_sb[:, p1 * 384:(p1 + 1) * 384], start=False, stop=True)
                    for p in range(4):
                        nc.vector.tensor_copy(stages[p][:, g * 384:(g + 1) * 384], pss[p][:])
                for p, (la, lb) in enumerate(PAIRS):
                    sv = stages[p][:].rearrange("p (g q d) -> q p g d", q=2, d=D_MODEL)
                    nc.sync.dma_start(out_pv[la, :, t0:t0 + G, :], sv[0])
                    nc.sync.dma_start(out_pv[lb, :, t0:t0 + G, :], sv[1])
                t0 += G

    nc.finalize()
    _NC_CACHE["nc"] = nc
    return nc


def _make_core_inputs(codes0, codes1, tabs_a, tabs_b):
    bc = np.zeros((2, 128), np.float32)
    bc[0, :] = 1.0
    bc[1, :] = -np.arange(128)
    bc = bc.astype(BF16)
    in_maps = []
    for c in range(N_CORES):
        lo_i, hi_i = c * PER_CORE, (c + 1) * PER_CORE
        c0 = np.zeros(N_PAD, np.int32)
        c1 = np.zeros(N_PAD, np.int32)
        c0[:PER_CORE] = codes0[lo_i:hi_i]
        c1[:PER_CORE] = codes1[lo_i:hi_i]
        il = np.stack([c0.reshape(N_TILES, P), c1.reshape(N_TILES, P)], axis=1)
        codes = np.ones((2, N_TILES * 256), np.float32)
        codes[0] = il.reshape(N_TILES * 256)
        in_maps.append({
            "codes": codes.astype(BF16),
            "bc": bc,
            "tabs_a": tabs_a,
            "tabs_b": tabs_b,
        })
    return in_maps


def _run_device(in_maps, trace=False):
    from concourse.bass_utils import run_bass_kernel_spmd
    nc = _build_nc()
    return run_bass_kernel_spmd(nc, in_maps, list(range(N_CORES)), trace=trace)


# ------------------------------------------------------------------- kernel
def kernel(voxel_feats, voxel_coors, W1, b1, gamma, beta, W2, b2, _trace=False, _device_only=False):
    voxel_feats = np.asarray(voxel_feats)
    coors = np.asarray(voxel_coors).astype(np.int64)
    W1, b1 = np.asarray(W1, np.float32), np.asarray(b1, np.float32)
    gamma, beta = np.asarray(gamma, np.float32), np.asarray(beta, np.float32)
    W2, b2 = np.asarray(W2, np.float32), np.asarray(b2, np.float32)

    wx = WIN_SHAPE[0]
    bwi0, ciw0 = _get_window_coors(coors, SHIFTS[0])
    bwi1, ciw1 = _get_window_coors(coors, SHIFTS[1])
    inds0, mask0 = _get_set_single_shift(bwi0, ciw0)
    inds1, mask1 = _get_set_single_shift(bwi1, ciw1)

    codes0 = (ciw0[:, 1] * wx + ciw0[:, 2]).astype(np.int32)
    codes1 = (ciw1[:, 1] * wx + ciw1[:, 2]).astype(np.int32)
    tables = _compute_tables(codes0, codes1, W1, b1, gamma, beta, W2, b2)
    tabs_a, tabs_b = _pack_tables(tables)

    in_maps = _make_core_inputs(codes0, codes1, tabs_a, tabs_b)
    res = _run_device(in_maps, trace=_trace)

    pos_embeds = np.empty((N_LAYERS, N_VOXELS, D_MODEL), np.float32)
    for c in range(N_CORES):
        pos_embeds[:, c * PER_CORE:(c + 1) * PER_CORE, :] = res.results[c]["out"][:, :PER_CORE, :]

    out = (voxel_feats, inds0, mask0, inds1, mask1, pos_embeds)
    if _trace or _device_only:
        return out, res
    return out


# revision 27
# speedup vs baseline: 2.3192x; 1.4521x over previous
"""DSVT input layer for Trainium2, 8 NeuronCores.

Outputs (matching reference):
  voxel_feats  : passthrough of the input array.
  inds/mask    : set-partition bookkeeping (argsort/unique/cumsum integer
                 math) — computed on host; ~0.5% of output bytes.
  pos_embeds   : [8, 100000, 192] f32 (614 MB, 99.4% of output bytes) —
                 computed on device. The window position embedding has only
                 12*12 = 144 distinct inputs, so each layer's embedding is a
                 144x192 table lookup. Each core takes 12500 voxels x all 8
                 layers: a one-hot(code) matrix is built on-chip (broadcast
                 matmul + is_equal) and multiplied against the tables
                 (bf16 hi+lo split for f32-exact results), then streamed to
                 DRAM in ~786KB batched DMAs. This keeps the kernel at the
                 HBM write roofline.
"""
import numpy as np
import ml_dtypes
from math import ceil

N_VOXELS = 100000
BATCH = 4
SPARSE_SHAPE = (468, 468, 1)
WIN_SHAPE = (12, 12, 1)
SET_SIZE = 36
N_LAYERS = 8
D_MODEL = 192
SHIFTS = ((0, 0, 0), (6, 6, 0))

N_CORES = 8
PER_CORE = N_VOXELS // N_CORES          # 12500
P = 128
N_TILES = ceil(PER_CORE / P)            # 98
N_PAD = N_TILES * P                     # 12544
G_MAX = 8                               # tiles per staged DMA batch
N_CODES = WIN_SHAPE[0] * WIN_SHAPE[1]   # 144
PAIRS = ((0, 2), (4, 6), (1, 3), (5, 7))  # layer pairs sharing a PSUM bank

BF16 = ml_dtypes.bfloat16


# ---------------------------------------------------------------- host: sets
def _get_window_coors(coors, shift):
    ssx, ssy, ssz = SPARSE_SHAPE
    wx, wy, wz = WIN_SHAPE
    mnx = ceil(ssx / wx) + 1
    mny = ceil(ssy / wy) + 1
    mnz = ceil(ssz / wz) + 1
    max_per_sample = mnx * mny * mnz
    sx, sy, sz = shift
    if ssz == wz:
        sz = 0
    scx = coors[:, 3] + sx
    scy = coors[:, 2] + sy
    scz = coors[:, 1] + sz
    batch_win = coors[:, 0] * max_per_sample + (scx // wx) * mny * mnz + (scy // wy) * mnz + (scz // wz)
    coors_in_win = np.stack([scz % wz, scy % wy, scx % wx], -1)
    return batch_win.astype(np.int64), coors_in_win.astype(np.int32)


def _get_inner_win_inds(win_inds):
    n = win_inds.shape[0]
    order = np.argsort(win_inds, kind="stable")
    s = win_inds[order]
    idx = np.arange(n, dtype=np.int32)
    is_start = np.concatenate([np.ones((1,), bool), s[1:] != s[:-1]])
    run_start = np.maximum.accumulate(np.where(is_start, idx, 0))
    inner = idx - run_start
    out = np.zeros(n, np.int32)
    out[order] = inner
    return out


def _get_set_single_shift(batch_win_inds, coors_in_win):
    wx, wy, wz = WIN_SHAPE
    max_voxel = wx * wy * wz
    n = batch_win_inds.shape[0]
    uniq, contiguous = np.unique(batch_win_inds, return_inverse=True)
    contiguous = contiguous.reshape(-1).astype(np.int32)
    win_num = int(uniq.shape[0])
    count = np.bincount(contiguous, minlength=win_num).astype(np.int32)
    setnum = -(-count // SET_SIZE)
    set_num = int(setnum.sum())
    set_win_inds = np.repeat(np.arange(win_num, dtype=np.int32), setnum)
    offs = np.concatenate([np.zeros(1, np.int64), np.cumsum(setnum)[:-1]]).astype(np.int32)
    set_inds_in_win = np.arange(set_num, dtype=np.int32) - offs[set_win_inds]
    base = set_inds_in_win[:, None] * SET_SIZE + np.arange(SET_SIZE, dtype=np.int32)[None, :]
    sel = (base * count[set_win_inds][:, None]) // (setnum[set_win_inds][:, None] * SET_SIZE)
    sel = sel + set_win_inds[:, None] * max_voxel

    inner = _get_inner_win_inds(contiguous)
    order1 = np.argsort(contiguous.astype(np.int64) * max_voxel + inner, kind="stable")
    c64 = contiguous.astype(np.int64)

    def partition(sort_key):
        order2 = np.argsort(sort_key, kind="stable")
        inner_sorted = np.zeros(n, np.int32)
        inner_sorted[order2] = inner[order1]
        pos_in_batch = inner_sorted.astype(np.int64) + max_voxel * c64
        padding = np.full((win_num * max_voxel,), -1, np.int32)
        padding[pos_in_batch] = np.arange(n, dtype=np.int32)
        return padding[sel]

    key_y = c64 * max_voxel + coors_in_win[:, 1] * wx * wz + coors_in_win[:, 2] * wz + coors_in_win[:, 0]
    key_x = c64 * max_voxel + coors_in_win[:, 2] * wy * wz + coors_in_win[:, 1] * wz + coors_in_win[:, 0]
    inds = np.stack([partition(key_y), partition(key_x)], 0).astype(np.int32)
    prefix = np.roll(inds, 1, axis=-1)
    prefix[:, :, 0] = -1
    mask = inds == prefix
    return inds, mask


# ------------------------------------------------------------- host: tables
def _compute_tables(codes0, codes1, W1, b1, gamma, beta, W2, b2):
    """144-row pos-embed table per layer; BN batch stats via code histograms."""
    wx, wy, _ = WIN_SHAPE
    n = codes0.shape[0]
    k = np.arange(N_CODES)
    loc_tab = np.stack([(k % wx) - wx / 2.0, (k // wx) - wy / 2.0], -1).astype(np.float32)
    counts = [np.bincount(codes0, minlength=N_CODES).astype(np.float64),
              np.bincount(codes1, minlength=N_CODES).astype(np.float64)]
    tables = np.zeros((N_LAYERS, N_CODES, D_MODEL), np.float32)
    for l in range(N_LAYERS):
        cnt = counts[l % 2]
        h = loc_tab @ W1[l] + b1[l]
        h64 = h.astype(np.float64)
        mu = (cnt[:, None] * h64).sum(0) / n
        var = (cnt[:, None] * (h64 - mu) ** 2).sum(0) / n
        mu32 = mu.astype(np.float32)
        var32 = var.astype(np.float32)
        hn = (h - mu32) / np.sqrt(var32 + 1e-5) * gamma[l] + beta[l]
        tables[l] = np.maximum(hn, 0.0) @ W2[l] + b2[l]
    return tables


def _pack_tables(tables):
    """Pack per-layer f32 tables into the SBUF-resident matmul rhs layouts.

    The gather matmuls run in float32r: exact for one-hot lhsT (the dropped
    lo*lo cross term is zero when one operand is exactly 1.0), and the cost
    model gives 1 cycle/row for moving dim >= 256 — so no bf16 hi/lo split
    is needed.

    tabs_a: [128, 4 pairs * 384] rows k=0..127
    tabs_b: [ 16, 4 pairs * 384] rows k=128..143
    where each 384 block is [layer_a 192 | layer_b 192].
    """
    hi = tables.astype(BF16).astype(np.float32)
    rounded = hi + (tables - hi).astype(BF16).astype(np.float32)  # fp32r grid
    tabs_a = np.zeros((128, 4, 2, D_MODEL), np.float32)
    # B table zero-padded to K=128: small-K (16) matmuls measured 3x slower
    # than K=128 on HW, and zero rows cost nothing extra.
    tabs_b = np.zeros((128, 4, 2, D_MODEL), np.float32)
    for p, (la, lb) in enumerate(PAIRS):
        for j, l in enumerate((la, lb)):
            tabs_a[:, p, j, :] = rounded[l, :128, :]
            tabs_b[:16, p, j, :] = rounded[l, 128:, :]
    return tabs_a.reshape(128, 4 * 2 * D_MODEL), tabs_b.reshape(128, 4 * 2 * D_MODEL)


# ------------------------------------------------------------- device kernel
_NC_CACHE = {}
import os as _os
_DENSE_TEST = bool(_os.environ.get("KERNEL_DENSE_TEST"))


def _build_nc():
    if "nc" in _NC_CACHE:
        return _NC_CACHE["nc"]
    import concourse.mybir as mybir
    from concourse.bacc import Bacc
    from concourse.tile import TileContext

    dt = mybir.dt
    # Bacc (not plain Bass): its generate_event_semaphores pass splits
    # multi-wait sync_info into the 1-wait-per-instruction form the TRN2
    # ISA requires — walrus rejects plain Bass Tile output for this kernel.
    nc = Bacc()
    # codes row 0: per-tile interleaved (128 shift0 | 128 shift1) voxel codes;
    # row 1: all-ones. bc row 0: ones, row 1: -partition_index. The broadcast
    # matmul bc.T @ codes then yields psum[p, v] = codes[v] - p, so the
    # one-hot compares need only immediate scalars (0 for table rows 0..127,
    # 128 for rows 128..143).
    codes_d = nc.declare_dram_parameter("codes", [2, N_TILES * 256], dt.bfloat16, isOutput=False)
    bc_d = nc.declare_dram_parameter("bc", [2, 128], dt.bfloat16, isOutput=False)
    tabs_a_d = nc.declare_dram_parameter("tabs_a", [128, 1536], dt.float32r, isOutput=False)
    tabs_b_d = nc.declare_dram_parameter("tabs_b", [128, 1536], dt.float32r, isOutput=False)
    out_d = nc.declare_dram_parameter("out", [N_LAYERS, N_PAD, D_MODEL], dt.float32, isOutput=True)

    with TileContext(nc) as tc:
        with (
            tc.tile_pool(name="const", bufs=1) as const_pool,
            tc.tile_pool(name="stage", bufs=2) as stage_pool,
            tc.tile_pool(name="oh", bufs=2) as oh_pool,
            tc.tile_pool(name="psum_bc", bufs=2, space="PSUM") as psum_bc_pool,
            tc.tile_pool(name="psum_out", bufs=6, space="PSUM") as psum_out_pool,
        ):
            codes_sb = const_pool.tile([2, N_TILES * 256], dt.bfloat16)
            nc.sync.dma_start(codes_sb[:], codes_d[:])
            bc_sb = const_pool.tile([2, 128], dt.bfloat16)
            nc.sync.dma_start(bc_sb[:], bc_d[:])
            tabs_a_sb = const_pool.tile([128, 1536], dt.float32r)
            nc.sync.dma_start(tabs_a_sb[:], tabs_a_d[:])
            tabs_b_sb = const_pool.tile([128, 1536], dt.float32r)
            nc.sync.dma_start(tabs_b_sb[:], tabs_b_d[:])

            # DRAM out viewed [layer, partition, tile, d] so the SBUF side of
            # the store keeps its partition dim first.
            out_pv = out_d[:].rearrange("l (t p) d -> l p t d", p=P)

            t0 = 0
            while t0 < N_TILES:
                G = min(G_MAX, N_TILES - t0)
                stages = [
                    stage_pool.tile([128, G * 384], dt.float32,
                                    name=f"stage{p}", tag=f"stage{p}")
                    for p in range(4)
                ]
                # Batched broadcast matmuls: K=2 matmuls pay a large small-K
                # penalty, so amortize them over up to 4 tiles (N=1024) each.
                ohA = oh_pool.tile([128, G * 256], dt.float32r, tag="ohA")
                ohB = oh_pool.tile([128, G * 256], dt.float32r, tag="ohB")
                c0 = 0
                while c0 < G:
                    C = min(2, G - c0)
                    psum_codes = psum_bc_pool.tile([128, C * 256], dt.float32, tag="bc")
                    nc.tensor.matmul(
                        psum_codes[:], bc_sb[:],
                        codes_sb[:, (t0 + c0) * 256:(t0 + c0 + C) * 256],
                        start=True, stop=True,
                    )
                    nc.vector.tensor_scalar(
                        ohA[:, c0 * 256:(c0 + C) * 256], psum_codes[:], 0.0, None,
                        mybir.AluOpType.is_equal,
                    )
                    nc.vector.tensor_scalar(
                        ohB[:, c0 * 256:(c0 + C) * 256], psum_codes[:], 128.0, None,
                        mybir.AluOpType.is_equal,
                    )
                    c0 += C
                for g in range(G):
                    # Same-lhsT runs (pairs 2p, 2p+1 share the shift-s one-hot).
                    pss = [psum_out_pool.tile([128, 384], dt.float32, tag="po",
                                              name=f"po{p}") for p in range(4)]
                    for sh in range(2):
                        lhsA = ohA[:, g * 256 + sh * 128:g * 256 + (sh + 1) * 128]
                        lhsB = ohB[:, g * 256 + sh * 128:g * 256 + (sh + 1) * 128]
                        p0, p1 = 2 * sh, 2 * sh + 1
                        nc.tensor.matmul(pss[p0][:], lhsA, tabs_a_sb[:, p0 * 384:(p0 + 1) * 384], start=True, stop=False)
                        nc.tensor.matmul(pss[p1][:], lhsA, tabs_a_sb[:, p1 * 384:(p1 + 1) * 384], start=True, stop=False)
                        nc.tensor.matmul(pss[p0][:], lhsB, tabs_b_sb[:, p0 * 384:(p0 + 1) * 384], start=False, stop=True)
                        nc.tensor.matmul(pss[p1][:], lhsB, tabs_b_sb[:, p1 * 384:(p1 + 1) * 384], start=False, stop=True)
                    # Split the PSUM drains: 2 on DVE, 2 on the idle ACT engine.
                    for p in range(4):
                        dst = stages[p][:, g * 384:(g + 1) * 384]
                        if p % 2 == 0:
                            nc.vector.tensor_copy(dst, pss[p][:])
                        else:
                            nc.scalar.copy(dst, pss[p][:])
                for p, (la, lb) in enumerate(PAIRS):
                    sv = stages[p][:].rearrange("p (g q d) -> q p g d", q=2, d=D_MODEL)
                    nc.sync.dma_start(out_pv[la, :, t0:t0 + G, :], sv[0])
                    nc.sync.dma_start(out_pv[lb, :, t0:t0 + G, :], sv[1])
                t0 += G

    nc.finalize()
    _NC_CACHE["nc"] = nc
    return nc


def _make_core_inputs(codes0, codes1, tabs_a, tabs_b):
    bc = np.zeros((2, 128), np.float32)
    bc[0, :] = 1.0
    bc[1, :] = -np.arange(128)
    bc = bc.astype(BF16)
    in_maps = []
    for c in range(N_CORES):
        lo_i, hi_i = c * PER_CORE, (c + 1) * PER_CORE
        c0 = np.zeros(N_PAD, np.int32)
        c1 = np.zeros(N_PAD, np.int32)
        c0[:PER_CORE] = codes0[lo_i:hi_i]
        c1[:PER_CORE] = codes1[lo_i:hi_i]
        il = np.stack([c0.reshape(N_TILES, P), c1.reshape(N_TILES, P)], axis=1)
        codes = np.ones((2, N_TILES * 256), np.float32)
        codes[0] = il.reshape(N_TILES * 256)
        in_maps.append({
            "codes": codes.astype(BF16),
            "bc": bc,
            "tabs_a": tabs_a,
            "tabs_b": tabs_b,
        })
    return in_maps


def _run_device(in_maps, trace=False):
    from concourse.bass_utils import run_bass_kernel_spmd
    nc = _build_nc()
    return run_bass_kernel_spmd(nc, in_maps, list(range(N_CORES)), trace=trace)


# ------------------------------------------------------------------- kernel
def kernel(voxel_feats, voxel_coors, W1, b1, gamma, beta, W2, b2, _trace=False, _device_only=False):
    voxel_feats = np.asarray(voxel_feats)
    coors = np.asarray(voxel_coors).astype(np.int64)
    W1, b1 = np.asarray(W1, np.float32), np.asarray(b1, np.float32)
    gamma, beta = np.asarray(gamma, np.float32), np.asarray(beta, np.float32)
    W2, b2 = np.asarray(W2, np.float32), np.asarray(b2, np.float32)

    wx = WIN_SHAPE[0]
    bwi0, ciw0 = _get_window_coors(coors, SHIFTS[0])
    bwi1, ciw1 = _get_window_coors(coors, SHIFTS[1])
    inds0, mask0 = _get_set_single_shift(bwi0, ciw0)
    inds1, mask1 = _get_set_single_shift(bwi1, ciw1)

    codes0 = (ciw0[:, 1] * wx + ciw0[:, 2]).astype(np.int32)
    codes1 = (ciw1[:, 1] * wx + ciw1[:, 2]).astype(np.int32)
    tables = _compute_tables(codes0, codes1, W1, b1, gamma, beta, W2, b2)
    tabs_a, tabs_b = _pack_tables(tables)

    in_maps = _make_core_inputs(codes0, codes1, tabs_a, tabs_b)
    res = _run_device(in_maps, trace=_trace)

    pos_embeds = np.empty((N_LAYERS, N_VOXELS, D_MODEL), np.float32)
    for c in range(N_CORES):
        pos_embeds[:, c * PER_CORE:(c + 1) * PER_CORE, :] = res.results[c]["out"][:, :PER_CORE, :]

    out = (voxel_feats, inds0, mask0, inds1, mask1, pos_embeds)
    if _trace or _device_only:
        return out, res
    return out


# revision 28
# speedup vs baseline: 2.3338x; 1.0063x over previous
"""DSVT input layer for Trainium2, 8 NeuronCores.

Outputs (matching reference):
  voxel_feats  : passthrough of the input array.
  inds/mask    : set-partition bookkeeping (argsort/unique/cumsum integer
                 math) — computed on host; ~0.5% of output bytes.
  pos_embeds   : [8, 100000, 192] f32 (614 MB, 99.4% of output bytes) —
                 computed on device. The window position embedding has only
                 12*12 = 144 distinct inputs, so each layer's embedding is a
                 144x192 table lookup. Each core takes 12500 voxels x all 8
                 layers: a one-hot(code) matrix is built on-chip (broadcast
                 matmul + is_equal) and multiplied against the tables
                 (bf16 hi+lo split for f32-exact results), then streamed to
                 DRAM in ~786KB batched DMAs. This keeps the kernel at the
                 HBM write roofline.
"""
import numpy as np
import ml_dtypes
from math import ceil

N_VOXELS = 100000
BATCH = 4
SPARSE_SHAPE = (468, 468, 1)
WIN_SHAPE = (12, 12, 1)
SET_SIZE = 36
N_LAYERS = 8
D_MODEL = 192
SHIFTS = ((0, 0, 0), (6, 6, 0))

N_CORES = 8
PER_CORE = N_VOXELS // N_CORES          # 12500
P = 128
N_TILES = ceil(PER_CORE / P)            # 98
N_PAD = N_TILES * P                     # 12544
G_MAX = 8                               # tiles per staged DMA batch
N_CODES = WIN_SHAPE[0] * WIN_SHAPE[1]   # 144
PAIRS = ((0, 2), (4, 6), (1, 3), (5, 7))  # layer pairs sharing a PSUM bank

BF16 = ml_dtypes.bfloat16


# ---------------------------------------------------------------- host: sets
def _get_window_coors(coors, shift):
    ssx, ssy, ssz = SPARSE_SHAPE
    wx, wy, wz = WIN_SHAPE
    mnx = ceil(ssx / wx) + 1
    mny = ceil(ssy / wy) + 1
    mnz = ceil(ssz / wz) + 1
    max_per_sample = mnx * mny * mnz
    sx, sy, sz = shift
    if ssz == wz:
        sz = 0
    scx = coors[:, 3] + sx
    scy = coors[:, 2] + sy
    scz = coors[:, 1] + sz
    batch_win = coors[:, 0] * max_per_sample + (scx // wx) * mny * mnz + (scy // wy) * mnz + (scz // wz)
    coors_in_win = np.stack([scz % wz, scy % wy, scx % wx], -1)
    return batch_win.astype(np.int64), coors_in_win.astype(np.int32)


def _get_inner_win_inds(win_inds):
    n = win_inds.shape[0]
    order = np.argsort(win_inds, kind="stable")
    s = win_inds[order]
    idx = np.arange(n, dtype=np.int32)
    is_start = np.concatenate([np.ones((1,), bool), s[1:] != s[:-1]])
    run_start = np.maximum.accumulate(np.where(is_start, idx, 0))
    inner = idx - run_start
    out = np.zeros(n, np.int32)
    out[order] = inner
    return out


def _get_set_single_shift(batch_win_inds, coors_in_win):
    wx, wy, wz = WIN_SHAPE
    max_voxel = wx * wy * wz
    n = batch_win_inds.shape[0]
    uniq, contiguous = np.unique(batch_win_inds, return_inverse=True)
    contiguous = contiguous.reshape(-1).astype(np.int32)
    win_num = int(uniq.shape[0])
    count = np.bincount(contiguous, minlength=win_num).astype(np.int32)
    setnum = -(-count // SET_SIZE)
    set_num = int(setnum.sum())
    set_win_inds = np.repeat(np.arange(win_num, dtype=np.int32), setnum)
    offs = np.concatenate([np.zeros(1, np.int64), np.cumsum(setnum)[:-1]]).astype(np.int32)
    set_inds_in_win = np.arange(set_num, dtype=np.int32) - offs[set_win_inds]
    base = set_inds_in_win[:, None] * SET_SIZE + np.arange(SET_SIZE, dtype=np.int32)[None, :]
    sel = (base * count[set_win_inds][:, None]) // (setnum[set_win_inds][:, None] * SET_SIZE)
    sel = sel + set_win_inds[:, None] * max_voxel

    inner = _get_inner_win_inds(contiguous)
    order1 = np.argsort(contiguous.astype(np.int64) * max_voxel + inner, kind="stable")
    c64 = contiguous.astype(np.int64)

    def partition(sort_key):
        order2 = np.argsort(sort_key, kind="stable")
        inner_sorted = np.zeros(n, np.int32)
        inner_sorted[order2] = inner[order1]
        pos_in_batch = inner_sorted.astype(np.int64) + max_voxel * c64
        padding = np.full((win_num * max_voxel,), -1, np.int32)
        padding[pos_in_batch] = np.arange(n, dtype=np.int32)
        return padding[sel]

    key_y = c64 * max_voxel + coors_in_win[:, 1] * wx * wz + coors_in_win[:, 2] * wz + coors_in_win[:, 0]
    key_x = c64 * max_voxel + coors_in_win[:, 2] * wy * wz + coors_in_win[:, 1] * wz + coors_in_win[:, 0]
    inds = np.stack([partition(key_y), partition(key_x)], 0).astype(np.int32)
    prefix = np.roll(inds, 1, axis=-1)
    prefix[:, :, 0] = -1
    mask = inds == prefix
    return inds, mask


# ------------------------------------------------------------- host: tables
def _compute_tables(codes0, codes1, W1, b1, gamma, beta, W2, b2):
    """144-row pos-embed table per layer; BN batch stats via code histograms."""
    wx, wy, _ = WIN_SHAPE
    n = codes0.shape[0]
    k = np.arange(N_CODES)
    loc_tab = np.stack([(k % wx) - wx / 2.0, (k // wx) - wy / 2.0], -1).astype(np.float32)
    counts = [np.bincount(codes0, minlength=N_CODES).astype(np.float64),
              np.bincount(codes1, minlength=N_CODES).astype(np.float64)]
    tables = np.zeros((N_LAYERS, N_CODES, D_MODEL), np.float32)
    for l in range(N_LAYERS):
        cnt = counts[l % 2]
        h = loc_tab @ W1[l] + b1[l]
        h64 = h.astype(np.float64)
        mu = (cnt[:, None] * h64).sum(0) / n
        var = (cnt[:, None] * (h64 - mu) ** 2).sum(0) / n
        mu32 = mu.astype(np.float32)
        var32 = var.astype(np.float32)
        hn = (h - mu32) / np.sqrt(var32 + 1e-5) * gamma[l] + beta[l]
        tables[l] = np.maximum(hn, 0.0) @ W2[l] + b2[l]
    return tables


def _pack_tables(tables):
    """Pack per-layer f32 tables into the SBUF-resident matmul rhs layouts.

    The gather matmuls run in float32r: exact for one-hot lhsT (the dropped
    lo*lo cross term is zero when one operand is exactly 1.0), and the cost
    model gives 1 cycle/row for moving dim >= 256 — so no bf16 hi/lo split
    is needed.

    tabs_a: [128, 4 pairs * 384] rows k=0..127
    tabs_b: [ 16, 4 pairs * 384] rows k=128..143
    where each 384 block is [layer_a 192 | layer_b 192].
    """
    hi = tables.astype(BF16).astype(np.float32)
    rounded = hi + (tables - hi).astype(BF16).astype(np.float32)  # fp32r grid
    tabs_a = np.zeros((128, 4, 2, D_MODEL), np.float32)
    # B table zero-padded to K=128: small-K (16) matmuls measured 3x slower
    # than K=128 on HW, and zero rows cost nothing extra.
    tabs_b = np.zeros((128, 4, 2, D_MODEL), np.float32)
    for p, (la, lb) in enumerate(PAIRS):
        for j, l in enumerate((la, lb)):
            tabs_a[:, p, j, :] = rounded[l, :128, :]
            tabs_b[:16, p, j, :] = rounded[l, 128:, :]
    return tabs_a.reshape(128, 4 * 2 * D_MODEL), tabs_b.reshape(128, 4 * 2 * D_MODEL)


# ------------------------------------------------------------- device kernel
_NC_CACHE = {}
import os as _os
_DENSE_TEST = bool(_os.environ.get("KERNEL_DENSE_TEST"))


def _build_nc():
    if "nc" in _NC_CACHE:
        return _NC_CACHE["nc"]
    import concourse.mybir as mybir
    from concourse.bacc import Bacc
    from concourse.tile import TileContext

    dt = mybir.dt
    # Bacc (not plain Bass): its generate_event_semaphores pass splits
    # multi-wait sync_info into the 1-wait-per-instruction form the TRN2
    # ISA requires — walrus rejects plain Bass Tile output for this kernel.
    nc = Bacc()
    # codes row 0: per-tile interleaved (128 shift0 | 128 shift1) voxel codes;
    # row 1: all-ones. bc row 0: ones, row 1: -partition_index. The broadcast
    # matmul bc.T @ codes then yields psum[p, v] = codes[v] - p, so the
    # one-hot compares need only immediate scalars (0 for table rows 0..127,
    # 128 for rows 128..143).
    codes_d = nc.declare_dram_parameter("codes", [2, N_TILES * 256], dt.bfloat16, isOutput=False)
    bc_d = nc.declare_dram_parameter("bc", [2, 128], dt.bfloat16, isOutput=False)
    tabs_a_d = nc.declare_dram_parameter("tabs_a", [128, 1536], dt.float32r, isOutput=False)
    tabs_b_d = nc.declare_dram_parameter("tabs_b", [128, 1536], dt.float32r, isOutput=False)
    out_d = nc.declare_dram_parameter("out", [N_LAYERS, N_PAD, D_MODEL], dt.float32, isOutput=True)

    with TileContext(nc) as tc:
        with (
            tc.tile_pool(name="const", bufs=1) as const_pool,
            tc.tile_pool(name="stage", bufs=2) as stage_pool,
            tc.tile_pool(name="oh", bufs=2) as oh_pool,
            tc.tile_pool(name="psum_bc", bufs=2, space="PSUM") as psum_bc_pool,
            tc.tile_pool(name="psum_out", bufs=6, space="PSUM") as psum_out_pool,
        ):
            codes_sb = const_pool.tile([2, N_TILES * 256], dt.bfloat16)
            nc.sync.dma_start(codes_sb[:], codes_d[:])
            bc_sb = const_pool.tile([2, 128], dt.bfloat16)
            nc.sync.dma_start(bc_sb[:], bc_d[:])
            tabs_a_sb = const_pool.tile([128, 1536], dt.float32r)
            nc.sync.dma_start(tabs_a_sb[:], tabs_a_d[:])
            tabs_b_sb = const_pool.tile([128, 1536], dt.float32r)
            nc.sync.dma_start(tabs_b_sb[:], tabs_b_d[:])

            # DRAM out viewed [layer, partition, tile, d] so the SBUF side of
            # the store keeps its partition dim first.
            out_pv = out_d[:].rearrange("l (t p) d -> l p t d", p=P)

            # First supergroup is small so the first stores issue after ~2
            # tiles of compute instead of 8 — trims the startup bubble on the
            # (bottleneck) store path. 2 + 12*8 = 98.
            sched = [2] + [G_MAX] * ((N_TILES - 2) // G_MAX)
            assert sum(sched) == N_TILES
            t0 = 0
            for G in sched:
                stages = [
                    stage_pool.tile([128, G * 384], dt.float32,
                                    name=f"stage{p}", tag=f"stage{p}")
                    for p in range(4)
                ]
                # Batched broadcast matmuls: K=2 matmuls pay a large small-K
                # penalty, so amortize them over up to 4 tiles (N=1024) each.
                ohA = oh_pool.tile([128, G * 256], dt.float32r, tag="ohA")
                ohB = oh_pool.tile([128, G * 256], dt.float32r, tag="ohB")
                c0 = 0
                while c0 < G:
                    C = min(2, G - c0)
                    psum_codes = psum_bc_pool.tile([128, C * 256], dt.float32, tag="bc")
                    nc.tensor.matmul(
                        psum_codes[:], bc_sb[:],
                        codes_sb[:, (t0 + c0) * 256:(t0 + c0 + C) * 256],
                        start=True, stop=True,
                    )
                    nc.vector.tensor_scalar(
                        ohA[:, c0 * 256:(c0 + C) * 256], psum_codes[:], 0.0, None,
                        mybir.AluOpType.is_equal,
                    )
                    nc.vector.tensor_scalar(
                        ohB[:, c0 * 256:(c0 + C) * 256], psum_codes[:], 128.0, None,
                        mybir.AluOpType.is_equal,
                    )
                    c0 += C
                for g in range(G):
                    # Same-lhsT runs (pairs 2p, 2p+1 share the shift-s one-hot).
                    pss = [psum_out_pool.tile([128, 384], dt.float32, tag="po",
                                              name=f"po{p}") for p in range(4)]
                    for sh in range(2):
                        lhsA = ohA[:, g * 256 + sh * 128:g * 256 + (sh + 1) * 128]
                        lhsB = ohB[:, g * 256 + sh * 128:g * 256 + (sh + 1) * 128]
                        p0, p1 = 2 * sh, 2 * sh + 1
                        nc.tensor.matmul(pss[p0][:], lhsA, tabs_a_sb[:, p0 * 384:(p0 + 1) * 384], start=True, stop=False)
                        nc.tensor.matmul(pss[p1][:], lhsA, tabs_a_sb[:, p1 * 384:(p1 + 1) * 384], start=True, stop=False)
                        nc.tensor.matmul(pss[p0][:], lhsB, tabs_b_sb[:, p0 * 384:(p0 + 1) * 384], start=False, stop=True)
                        nc.tensor.matmul(pss[p1][:], lhsB, tabs_b_sb[:, p1 * 384:(p1 + 1) * 384], start=False, stop=True)
                    # Split the PSUM drains: 2 on DVE, 2 on the idle ACT engine.
                    for p in range(4):
                        dst = stages[p][:, g * 384:(g + 1) * 384]
                        if p % 2 == 0:
                            nc.vector.tensor_copy(dst, pss[p][:])
                        else:
                            nc.scalar.copy(dst, pss[p][:])
                for p, (la, lb) in enumerate(PAIRS):
                    sv = stages[p][:].rearrange("p (g q d) -> q p g d", q=2, d=D_MODEL)
                    nc.sync.dma_start(out_pv[la, :, t0:t0 + G, :], sv[0])
                    nc.sync.dma_start(out_pv[lb, :, t0:t0 + G, :], sv[1])
                t0 += G

    nc.finalize()
    _NC_CACHE["nc"] = nc
    return nc


def _make_core_inputs(codes0, codes1, tabs_a, tabs_b):
    bc = np.zeros((2, 128), np.float32)
    bc[0, :] = 1.0
    bc[1, :] = -np.arange(128)
    bc = bc.astype(BF16)
    in_maps = []
    for c in range(N_CORES):
        lo_i, hi_i = c * PER_CORE, (c + 1) * PER_CORE
        c0 = np.zeros(N_PAD, np.int32)
        c1 = np.zeros(N_PAD, np.int32)
        c0[:PER_CORE] = codes0[lo_i:hi_i]
        c1[:PER_CORE] = codes1[lo_i:hi_i]
        il = np.stack([c0.reshape(N_TILES, P), c1.reshape(N_TILES, P)], axis=1)
        codes = np.ones((2, N_TILES * 256), np.float32)
        codes[0] = il.reshape(N_TILES * 256)
        in_maps.append({
            "codes": codes.astype(BF16),
            "bc": bc,
            "tabs_a": tabs_a,
            "tabs_b": tabs_b,
        })
    return in_maps


def _run_device(in_maps, trace=False):
    from concourse.bass_utils import run_bass_kernel_spmd
    nc = _build_nc()
    return run_bass_kernel_spmd(nc, in_maps, list(range(N_CORES)), trace=trace)


# ------------------------------------------------------------------- kernel
def kernel(voxel_feats, voxel_coors, W1, b1, gamma, beta, W2, b2, _trace=False, _device_only=False):
    voxel_feats = np.asarray(voxel_feats)
    coors = np.asarray(voxel_coors).astype(np.int64)
    W1, b1 = np.asarray(W1, np.float32), np.asarray(b1, np.float32)
    gamma, beta = np.asarray(gamma, np.float32), np.asarray(beta, np.float32)
    W2, b2 = np.asarray(W2, np.float32), np.asarray(b2, np.float32)

    wx = WIN_SHAPE[0]
    bwi0, ciw0 = _get_window_coors(coors, SHIFTS[0])
    bwi1, ciw1 = _get_window_coors(coors, SHIFTS[1])
    inds0, mask0 = _get_set_single_shift(bwi0, ciw0)
    inds1, mask1 = _get_set_single_shift(bwi1, ciw1)

    codes0 = (ciw0[:, 1] * wx + ciw0[:, 2]).astype(np.int32)
    codes1 = (ciw1[:, 1] * wx + ciw1[:, 2]).astype(np.int32)
    tables = _compute_tables(codes0, codes1, W1, b1, gamma, beta, W2, b2)
    tabs_a, tabs_b = _pack_tables(tables)

    in_maps = _make_core_inputs(codes0, codes1, tabs_a, tabs_b)
    res = _run_device(in_maps, trace=_trace)

    pos_embeds = np.empty((N_LAYERS, N_VOXELS, D_MODEL), np.float32)
    for c in range(N_CORES):
        pos_embeds[:, c * PER_CORE:(c + 1) * PER_CORE, :] = res.results[c]["out"][:, :PER_CORE, :]

    out = (voxel_feats, inds0, mask0, inds1, mask1, pos_embeds)
    if _trace or _device_only:
        return out, res
    return out
